# revision 1
# baseline (speedup 1.0000x reference)
"""Trainium2 Bass kernel for batched masked attention.

Problem: q,k,v [16, 2048, 256] f32, mask [16, 2048, 2048] int32.
  scores = (q @ k^T) / 16
  scores = where(mask == 0, 0.0, scores)      # NOT -inf
  att    = softmax(scores, axis=-1)
  att    = 0 if mask.sum() == 0 (handled host-side)
  out    = att @ v

Sharding: batch dim across 8 NeuronCores (2 batches per core); each core
computes full attention for its batches independently; host gathers.

The host pre-arranges inputs into the exact on-chip layouts (all free — the
kernel owns its input contract):
  qt/kt : [BPC, 128, D/128, S] f32 — head-dim on partitions (q/k transposed)
  vp    : [BPC, 128, S/128, D+2] f32 — v tiles with two ones columns; in the
          output matmul the ones column accumulates the softmax denominator Z
  mask8 : [BPC, 4, 128, S/128, 512] u8 — mask transposed (key-major) and cast
          to u8, pre-tiled per 512-query chunk
Everything is computed in the transposed score domain so no on-chip
transposes are needed at all; per 512-query chunk:
  mm1 (PE, f32r 1cyc/row): sT[128 key, 512 qry] = kT.T @ qT   (K=256, 2 psum accums)
  DVE in-place:            sT = (sT * 1/16) * mask8           (u8 mask)
  ACT:                     attT = exp(sT)  PSUM->SBUF, rounded to f32r
  mm2 (PE, f32r):          out[128 qry, 258] += attT.T @ v'   (16 accums)
  DVE: 1/Z + scale-copy -> out tile -> DMA
mm2 for chunk ic-1 is emitted after mm1 of chunk ic (software pipelining) so
the PE never idles on the DVE/ACT epilogue; batch-level loads ride the
gpsimd SWDGE ring to keep the sync ring free for mask/out streaming.
"""

import sys

if "/opt/trn_rl_repo" not in sys.path:
    sys.path.insert(0, "/opt/trn_rl_repo")

from contextlib import ExitStack

import numpy as np

import concourse.mybir as mybir
import concourse.tile as tile
from concourse import bacc
from concourse.bass_utils import run_bass_kernel_spmd

B, S, D = 16, 2048, 256
NCORES = 8
BPC = B // NCORES  # batches per core
P = 128
QT = S // P        # 16 key blocks of 128
IC = S // 512      # 4 query chunks of 512
KC = D // P        # 2 contraction chunks of 128
SCALE = 1.0 / 16.0  # 1/sqrt(D)

F32 = mybir.dt.float32
F32R = mybir.dt.float32r
U8 = mybir.dt.uint8


def build_program(reps=1):
    nc = bacc.Bacc("TRN2", target_bir_lowering=False, debug=False)
    qtd = nc.dram_tensor("qt", [BPC, P, KC, S], F32R, kind="ExternalInput").ap()
    ktd = nc.dram_tensor("kt", [BPC, P, KC, S], F32R, kind="ExternalInput").ap()
    vpd = nc.dram_tensor("vp", [BPC, P, QT, D + 2], F32R, kind="ExternalInput").ap()
    m8d = nc.dram_tensor("mask8", [BPC, IC, P, QT, 512], U8, kind="ExternalInput").ap()
    out = nc.dram_tensor("out", [BPC, S, D], F32, kind="ExternalOutput").ap()

    with tile.TileContext(nc) as tc, ExitStack() as ctx:
        kt_pool = ctx.enter_context(tc.tile_pool(name="kt", bufs=2))
        qt_pool = ctx.enter_context(tc.tile_pool(name="qt", bufs=2))
        vp_pool = ctx.enter_context(tc.tile_pool(name="vp", bufs=2))
        mask_pool = ctx.enter_context(tc.tile_pool(name="maskp", bufs=3))
        att_pool = ctx.enter_context(tc.tile_pool(name="att", bufs=2))
        osb_pool = ctx.enter_context(tc.tile_pool(name="osb", bufs=4))
        rec_pool = ctx.enter_context(tc.tile_pool(name="rec", bufs=4))
        # ps_s tiles span 2 PSUM banks (a PAIR of key blocks) so one DVE op
        # and one ACT exp cover 1024 columns, halving their per-op overhead
        ps_s = ctx.enter_context(tc.tile_pool(name="ps_s", bufs=3, space="PSUM"))
        ps_out = ctx.enter_context(tc.tile_pool(name="ps_out", bufs=2, space="PSUM"))

        def build_inputs(b):
            # chunked loads so each mm1 only waits for the slices it reads
            # (Tile tracks sub-tile AP ranges)
            kt = kt_pool.tile([P, KC, S], F32R, tag="kt")
            qt = qt_pool.tile([P, KC, S], F32R, tag="qt")
            nc.gpsimd.dma_start(qt[:, :, :512], qtd[b][:, :, :512])
            for jb in range(4):
                nc.gpsimd.dma_start(
                    kt[:, :, jb * P : (jb + 1) * P],
                    ktd[b][:, :, jb * P : (jb + 1) * P],
                )
            for c in range(1, IC):
                nc.gpsimd.dma_start(
                    kt[:, :, c * 512 : (c + 1) * 512],
                    ktd[b][:, :, c * 512 : (c + 1) * 512],
                )
            for c in range(1, IC):
                nc.gpsimd.dma_start(
                    qt[:, :, c * 512 : (c + 1) * 512],
                    qtd[b][:, :, c * 512 : (c + 1) * 512],
                )
            vp = vp_pool.tile([P, QT, D + 2], F32R, tag="vp")
            nc.gpsimd.dma_start(vp[:], vpd[b])
            return kt, qt, vp

        def mm1_group(b, ic, g, kt, qt, mt, att):
            """scoresT + mask + exp for key blocks 4g..4g+3 of query chunk ic."""
            for jp in range(2 * g, 2 * g + 2):  # pairs of key blocks
                ps = ps_s.tile([P, 1024], F32, tag="score")
                for half in range(2):
                    jb = 2 * jp + half
                    for kc in range(KC):
                        nc.tensor.matmul(
                            ps[:, half * 512 : (half + 1) * 512],
                            lhsT=kt[:, kc, jb * P : (jb + 1) * P],
                            rhs=qt[:, kc, ic * 512 : (ic + 1) * 512],
                            start=(kc == 0),
                            stop=(kc == KC - 1),
                        )
                nc.vector.scalar_tensor_tensor(
                    out=ps[:],
                    in0=ps[:],
                    scalar=SCALE,
                    in1=mt[:, 2 * jp : 2 * jp + 2, :],
                    op0=mybir.AluOpType.mult,
                    op1=mybir.AluOpType.mult,
                )
                nc.scalar.activation(
                    att[:, 2 * jp : 2 * jp + 2, :],
                    ps[:],
                    mybir.ActivationFunctionType.Exp,
                )

        def mm2_group(b, ic, att, vp, iq):
            """att.T @ v' + normalize + store for query tile iq of chunk ic."""
            po = ps_out.tile([P, D + 2], F32, tag="ps_out")
            for jb in range(QT):
                nc.tensor.matmul(
                    po[:],
                    lhsT=att[:, jb, iq * P : (iq + 1) * P],
                    rhs=vp[:, jb, :],
                    start=(jb == 0),
                    stop=(jb == QT - 1),
                )
            rec = rec_pool.tile([P, 1], F32, tag="rec")
            nc.vector.reciprocal(rec[:], po[:, D : D + 1])
            osb = osb_pool.tile([P, D], F32, tag="osb")
            nc.scalar.activation(
                osb[:],
                po[:, :D],
                mybir.ActivationFunctionType.Copy,
                scale=rec[:],
            )
            it = ic * 4 + iq
            nc.sync.dma_start(out[b, it * P : (it + 1) * P, :], osb[:])

        # Software-pipelined emission: mm2 groups for chunk ic-1 interleave
        # with mm1 groups for chunk ic, so the PE never waits on the DVE/ACT
        # epilogue; next batch's loads are emitted mid-batch for prefetch.
        batches = [b for _ in range(reps) for b in range(BPC)]
        # PE warm-up: ~4us of dummy matmuls during the initial DMA wait so
        # the HAM clock gate is at 2.4 GHz when real work arrives.
        warm = mask_pool.tile([P, 512], F32, tag="warm")
        nc.gpsimd.memset(warm[:], 0.0)
        for i in range(4):
            wp = ps_out.tile([P, 512], F32, tag="ps_out")
            nc.tensor.matmul(
                wp[:], lhsT=warm[:, :P], rhs=warm[:], start=True, stop=True
            )
        inputs = {0: build_inputs(batches[0])}
        pending = None
        for idx, b in enumerate(batches):
            kt, qt, vp = inputs.pop(idx)
            for ic in range(IC):
                mt = mask_pool.tile([P, QT, 512], U8, tag="maskt")
                if idx == 0 and ic == 0:
                    # split the first mask load so STT on key block 0 starts
                    # after 256KB instead of 1MB
                    for g4 in range(4):
                        nc.sync.dma_start(
                            mt[:, g4 * 4 : (g4 + 1) * 4, :],
                            m8d[b, ic, :, g4 * 4 : (g4 + 1) * 4, :],
                        )
                else:
                    nc.sync.dma_start(mt[:], m8d[b, ic])
                att = att_pool.tile([P, QT, 512], F32R, tag="att")
                for g in range(4):
                    mm1_group(b, ic, g, kt, qt, mt, att)
                    if pending is not None:
                        mm2_group(*pending, iq=g)
                if ic == 1 and idx + 1 < len(batches):
                    inputs[idx + 1] = build_inputs(batches[idx + 1])
                pending = (b, ic, att, vp)
        for g in range(4):
            mm2_group(*pending, iq=g)

    nc.compile()
    return nc


def prep_inputs(q, k, v, mask):
    """Host-side layout prep; returns per-core in_maps."""
    q = np.asarray(q, dtype=np.float32)
    k = np.asarray(k, dtype=np.float32)
    v = np.asarray(v, dtype=np.float32)
    # [B, S, D] -> [B, P, KC, S]  (transposed, head-dim on partitions)
    qt = np.ascontiguousarray(
        q.transpose(0, 2, 1).reshape(B, KC, P, S).transpose(0, 2, 1, 3)
    )
    kt = np.ascontiguousarray(
        k.transpose(0, 2, 1).reshape(B, KC, P, S).transpose(0, 2, 1, 3)
    )
    # [B, S, D] -> [B, P, QT, D+2] with ones in the last two columns
    vp = np.ones((B, P, QT, D + 2), dtype=np.float32)
    vp[..., :D] = v.reshape(B, QT, P, D).transpose(0, 2, 1, 3)
    # mask [B, S(query), S(key)] -> u8 tiles [B, IC, P(key), QT, 512(query)]
    m8 = np.ascontiguousarray(
        (np.asarray(mask) != 0)
        .astype(np.uint8)
        .reshape(B, IC, 512, QT, P)
        .transpose(0, 1, 4, 3, 2)
    )
    return [
        {
            "qt": qt[c * BPC : (c + 1) * BPC],
            "kt": kt[c * BPC : (c + 1) * BPC],
            "vp": vp[c * BPC : (c + 1) * BPC],
            "mask8": m8[c * BPC : (c + 1) * BPC],
        }
        for c in range(NCORES)
    ]


_NC_CACHE = None


def _get_program():
    global _NC_CACHE
    if _NC_CACHE is None:
        _NC_CACHE = build_program()
    return _NC_CACHE


def kernel(q, k, v, mask):
    mask = np.asarray(mask)
    if mask.sum() == 0:
        return np.zeros((B, S, D), dtype=np.float32)
    nc = _get_program()
    in_maps = prep_inputs(q, k, v, mask)
    res = run_bass_kernel_spmd(nc, in_maps, list(range(NCORES)))
    return np.concatenate([res.results[c]["out"] for c in range(NCORES)], axis=0)

